# revision 23
# baseline (speedup 1.0000x reference)
"""GCNN message-passing kernel for 8 Trainium2 NeuronCores.

Sharding: adjacency / atom-feature rows are sharded across 8 cores
(1024 atoms each).  h (node features) is replicated via bf16 AllGather
after every layer; the small readout MLP is computed redundantly on
every core after an AllReduce of the per-shard atom sums.

The masked attention  M = A * sigmoid(s_self[i] + s_nei[j] + b)  is
computed without a separate mask multiply:  since A is binary,
    M^T[j,i] = sigmoid( 48*(A^T[j,i] - 1 + (s_self[i]+b)/48) + s_nei[j] )
(sigmoid(-48) ~ 1e-21 == 0).  Layer 0 transposes A tiles on the PE
(matmul with identity) and stores  C' = A^T - 1  (exact in fp16) to DRAM;
this pipeline depends only on A so it overlaps the input layer and first
AllGather.  Every layer then computes  sigmoid(48*(C' + pre_d) + s_nei)
where pre_d[i] = (s_self_d[i]+b)/48 is a broadcast row.

The aggregation runs transposed:  agg^T[h',i] = sum_j h[j,h'] * M^T[j,i]
so the gathered h is the PE's stationary operand (natural layout) and
nothing but the binary A ever needs a transpose.
"""

import numpy as np

import concourse.bass as bass
import concourse.bacc as bacc
import concourse.mybir as mybir
import concourse.tile as tile
from concourse.bass_utils import run_bass_kernel_spmd
from concourse.masks import make_identity

FP32 = mybir.dt.float32
BF16 = mybir.dt.bfloat16
FP16 = mybir.dt.float16
AF = mybir.ActivationFunctionType
ALU = mybir.AluOpType

MASK = 48.0  # sigmoid(-48) ~ 1.4e-21; 48 and 1/48-scaled terms exact enough in fp16

P = 128


def build_gcnn(N=8192, F=133, H=256, MF=200, RH=512, DEPTH=3, N_RO=2, n_cores=8,
               ach=None, ic=None, no_collectives=False):
    S = N // n_cores          # atoms per core (row shard)
    JT = N // P               # j (neighbor) tiles
    IT = S // P               # i (own atom) tiles
    IC = ic or (512 if S % 512 == 0 else S)   # i-chunk = psum free dim
    NIC = S // IC
    HC = H // P               # hidden chunks of 128
    PAY = H + 2               # payload row: h | s_nei | pad
    ACH = ach or min(2048, N)  # adjacency column chunk per SBUF load
    NJH = N // ACH
    JL = ACH // P             # j tiles per column chunk
    FT = (F + P - 1) // P
    RT = RH // P
    rg = [list(range(n_cores))]

    nc = bacc.Bacc("TRN2", target_bir_lowering=False, debug=False,
                   num_devices=n_cores)

    # ---------------- I/O ----------------
    A = nc.dram_tensor("adj", [S, N], FP32, kind="ExternalInput")
    X = nc.dram_tensor("atom", [S, F], FP32, kind="ExternalInput")
    MOLV = nc.dram_tensor("mol", [1, MF], FP32, kind="ExternalInput")
    WIN = nc.dram_tensor("w_in", [F, H], FP32, kind="ExternalInput")
    BIN = nc.dram_tensor("b_in", [1, H], FP32, kind="ExternalInput")
    WATT = nc.dram_tensor("w_att", [2, H], FP32, kind="ExternalInput")
    BATT = nc.dram_tensor("b_att", [1, 1], FP32, kind="ExternalInput")
    WNODE = nc.dram_tensor("w_node", [DEPTH, H, H], FP32, kind="ExternalInput")
    BNODE = nc.dram_tensor("b_node", [1, DEPTH * H], FP32, kind="ExternalInput")
    WROIN = nc.dram_tensor("w_ro_in", [H + MF, RH], FP32, kind="ExternalInput")
    BROIN = nc.dram_tensor("b_ro_in", [1, RH], FP32, kind="ExternalInput")
    WROH = nc.dram_tensor("w_ro_hid", [N_RO, RH, RH], FP32, kind="ExternalInput")
    BROH = nc.dram_tensor("b_ro_hid", [1, N_RO * RH], FP32, kind="ExternalInput")
    WOUT = nc.dram_tensor("w_out", [RH, 1], FP32, kind="ExternalInput")
    BOUT = nc.dram_tensor("b_out", [1, 1], FP32, kind="ExternalInput")
    OUT = nc.dram_tensor("out", [1, 1], FP32, kind="ExternalOutput")

    with tile.TileContext(nc) as tc:
        _build_body(nc, tc, locals())
    nc.compile()
    return nc


def _build_body(nc, tc, v):
    N, F, H, MF, RH, DEPTH, N_RO = (v[k] for k in
                                    ("N", "F", "H", "MF", "RH", "DEPTH", "N_RO"))
    S, JT, IT, IC, NIC, HC, PAY = (v[k] for k in
                                   ("S", "JT", "IT", "IC", "NIC", "HC", "PAY"))
    ACH, NJH, JL, FT, RT, rg = (v[k] for k in
                                ("ACH", "NJH", "JL", "FT", "RT", "rg"))
    A, X, MOLV, WIN, BIN, WATT, BATT = (v[k] for k in
                                        ("A", "X", "MOLV", "WIN", "BIN", "WATT", "BATT"))
    WNODE, BNODE, WROIN, BROIN, WROH, BROH, WOUT, BOUT, OUT = (
        v[k] for k in ("WNODE", "BNODE", "WROIN", "BROIN", "WROH", "BROH",
                       "WOUT", "BOUT", "OUT"))

    import contextlib
    ctx = contextlib.ExitStack()
    with ctx:
        consts = ctx.enter_context(tc.tile_pool(name="consts", bufs=1))
        work = ctx.enter_context(tc.tile_pool(name="work", bufs=3))
        big = ctx.enter_context(tc.tile_pool(name="big", bufs=1))
        aload = ctx.enter_context(tc.tile_pool(name="aload", bufs=2))
        pacc = ctx.enter_context(tc.tile_pool(name="pacc", bufs=1, space="PSUM"))
        pcyc = ctx.enter_context(tc.tile_pool(name="pcyc", bufs=3, space="PSUM"))
        dram = ctx.enter_context(tc.tile_pool(name="dram", bufs=1, space="DRAM"))

        # ------------- DRAM scratch -------------
        T0 = dram.tile([P, JT, S], FP16, name="t0_mask")
        pays = [dram.tile([P, IT * PAY], BF16, name=f"pay{d}")
                for d in range(DEPTH)]
        Gs = [dram.tile([v["n_cores"] * P, IT * PAY], BF16,
                        addr_space="Shared", name=f"gath{d}")
              for d in range(DEPTH)]
        ar_in = dram.tile([1, H], FP32, name="ar_in")
        ar_out = dram.tile([1, H], FP32, addr_space="Shared", name="ar_out")

        # ------------- constants / weights -------------
        ones_bf = consts.tile([P, P], BF16, name="ones_bf")
        nc.gpsimd.memset(ones_bf[:], 1.0)
        ones_f = consts.tile([P, P], FP32, name="ones_f")
        nc.gpsimd.memset(ones_f[:], 1.0)
        ident_bf = consts.tile([P, P], BF16, name="ident_bf")
        make_identity(nc, ident_bf[:])
        ident_f = consts.tile([P, P], FP32, name="ident_f")
        make_identity(nc, ident_f[:])
        neg1_col = consts.tile([P, 1], FP32, name="neg1_col")
        nc.gpsimd.memset(neg1_col[:], -1.0)

        win_sb = consts.tile([P, FT, H], FP32, name="win_sb")
        nc.sync.dma_start(out=win_sb[:, 0, :], in_=WIN[0:P, :])
        if F > P:
            nc.sync.dma_start(out=win_sb[0:F - P, 1, :], in_=WIN[P:F, :])
        bin_f = consts.tile([1, H], FP32, name="bin_f")
        nc.sync.dma_start(out=bin_f[:], in_=BIN[:])
        wnode_sb = consts.tile([P, DEPTH, HC, H], FP32, name="wnode_sb")
        nc.sync.dma_start(out=wnode_sb[:],
                          in_=WNODE.rearrange("d (kt p) h -> p d kt h", p=P))
        bnode_f = consts.tile([1, DEPTH * H], FP32, name="bnode_f")
        nc.sync.dma_start(out=bnode_f[:], in_=BNODE[:])

        watt_sb = consts.tile([1, 2 * H], FP32, name="watt_sb")
        nc.sync.dma_start(out=watt_sb[0:1, 0:H], in_=WATT[0:1, :])
        nc.sync.dma_start(out=watt_sb[0:1, H:2 * H], in_=WATT[1:2, :])
        batt_sb = consts.tile([1, 1], FP32, name="batt_sb")
        nc.sync.dma_start(out=batt_sb[:], in_=BATT[:])

        # readout weights, fp32
        wro_sb = consts.tile([P, 4, RH], FP32, name="wro_sb")
        nc.vector.memset(wro_sb[:], 0.0)
        nc.sync.dma_start(out=wro_sb[:, 0:2, :],
                          in_=WROIN[0:2 * P, :].rearrange("(t p) r -> p t r", p=P))
        nkm = (H + MF) - 2 * P  # rows of W_ro_in fed by mol features = MF
        full_mol_t = MF // P
        nc.sync.dma_start(
            out=wro_sb[:, 2:2 + full_mol_t, :],
            in_=WROIN[2 * P:2 * P + full_mol_t * P, :].rearrange(
                "(t p) r -> p t r", p=P))
        rem = nkm - full_mol_t * P
        if rem:
            nc.sync.dma_start(out=wro_sb[0:rem, 2 + full_mol_t, :],
                              in_=WROIN[2 * P + full_mol_t * P:, :])
        broin_sb = consts.tile([1, RH], FP32, name="broin_sb")
        nc.sync.dma_start(out=broin_sb[:], in_=BROIN[:])
        wroh_sb = consts.tile([P, N_RO, RT, RH], FP32, name="wroh_sb")
        nc.sync.dma_start(out=wroh_sb[:],
                          in_=WROH.rearrange("d (t p) r -> p d t r", p=P))
        broh_sb = consts.tile([1, N_RO * RH], FP32, name="broh_sb")
        nc.sync.dma_start(out=broh_sb[:], in_=BROH[:])
        wout_sb = consts.tile([P, RT, 1], FP32, name="wout_sb")
        nc.sync.dma_start(out=wout_sb[:], in_=WOUT.rearrange("(t p) o -> p t o", p=P))
        bout_sb = consts.tile([1, 1], FP32, name="bout_sb")
        nc.sync.dma_start(out=bout_sb[:], in_=BOUT[:])
        mol_sb = consts.tile([1, MF], FP32, name="mol_sb")
        nc.sync.dma_start(out=mol_sb[:], in_=MOLV[:])

        # b_att broadcast to a per-partition column
        p_b = pcyc.tile([P, 1], FP32, tag="ps")
        nc.tensor.matmul(p_b[:], lhsT=ones_f[0:1, :], rhs=batt_sb[:],
                         start=True, stop=True)
        batt_col = consts.tile([P, 1], FP32, name="batt_col")
        nc.any.tensor_copy(out=batt_col[:], in_=p_b[:])

        # W_att rows broadcast across partitions (bf16)
        wa_bc = []
        for a in range(2):
            p_w = pcyc.tile([P, H], FP32, tag="ps")
            nc.tensor.matmul(p_w[:], lhsT=ones_f[0:1, :],
                             rhs=watt_sb[0:1, a * H:(a + 1) * H],
                             start=True, stop=True)
            t = consts.tile([P, H], FP32, name=f"wa_bc{a}")
            nc.any.tensor_copy(out=t[:], in_=p_w[:])
            wa_bc.append(t)

        # persistent self-score columns per produced h (b_att folded in)
        s_cols = [consts.tile([P, IT], FP32, name=f"s_self{d}")
                  for d in range(DEPTH + 1)]

        # --------- shared helper: finish h tiles -> scores + payload + AG ---------
        def finish_h(d_prod, h_all, snei_col):
            """h_all[:, it, 0:H] already written (bf16).  Computes s_self/s_nei,
            stashes s_nei into payload col H, folds b_att into s_self, then
            payload DMA + AllGather (except after the last layer)."""
            for it in range(IT):
                scr = work.tile([P, H], BF16, tag="s_scr")
                nc.vector.scalar_tensor_tensor(
                    out=scr[:], in0=h_all[:, it, 0:H], scalar=1.0,
                    in1=wa_bc[1][:], op0=ALU.mult, op1=ALU.mult,
                    accum_out=s_cols[d_prod][:, it:it + 1])
                scr2 = work.tile([P, H], BF16, tag="s_scr")
                nc.vector.scalar_tensor_tensor(
                    out=scr2[:], in0=h_all[:, it, 0:H], scalar=1.0,
                    in1=wa_bc[0][:], op0=ALU.mult, op1=ALU.mult,
                    accum_out=snei_col[:, it:it + 1])
                nc.vector.tensor_copy(out=h_all[:, it, H:H + 1],
                                      in_=snei_col[:, it:it + 1])
            nc.vector.tensor_scalar_add(s_cols[d_prod][:], s_cols[d_prod][:],
                                        batt_col[:, 0:1])
            if d_prod < DEPTH:
                nc.sync.dma_start(
                    out=pays[d_prod][:].rearrange("p (t c) -> p t c", t=IT),
                    in_=h_all[:])
                if v["no_collectives"]:
                    nc.sync.dma_start(out=Gs[d_prod][0:P, :],
                                      in_=pays[d_prod][:])
                else:
                    nc.gpsimd.collective_compute(
                        "AllGather", ALU.bypass, replica_groups=rg,
                        ins=[pays[d_prod][:].opt()], outs=[Gs[d_prod][:].opt()])

        # ---------------- input layer: h0 = X @ W_in + b_in ----------------
        x_sb = big.tile([P, IT, F], FP32, name="x_sb")
        nc.sync.dma_start(out=x_sb[:],
                          in_=X.rearrange("(t p) f -> p t f", p=P))
        h_all = big.tile([P, IT, PAY], BF16, tag="h_all")
        snei_col = big.tile([P, IT], FP32, tag="snei_col")
        for it in range(IT):
            # transpose X tile chunks on the PE
            xt = []
            for c in range(FT):
                fw = min(P, F - c * P)
                p_xt = pcyc.tile([P, P], FP32, tag="ps")
                nc.tensor.matmul(p_xt[0:fw, :],
                                 lhsT=x_sb[:, it, c * P:c * P + fw],
                                 rhs=ident_f[:], start=True, stop=True)
                xt_sb = work.tile([P, P], FP32, tag="xt_sb")
                nc.any.tensor_copy(out=xt_sb[0:fw, :], in_=p_xt[0:fw, :])
                xt.append((xt_sb, fw))
            p_h0 = pcyc.tile([P, H], FP32, tag="ps")
            for c, (xt_sb, fw) in enumerate(xt):
                nc.tensor.matmul(p_h0[:], lhsT=xt_sb[0:fw, :],
                                 rhs=win_sb[0:fw, c, :],
                                 start=(c == 0), stop=False)
            nc.tensor.matmul(p_h0[:], lhsT=ones_f[0:1, :], rhs=bin_f[:],
                             start=False, stop=True)
            nc.any.tensor_copy(out=h_all[:, it, 0:H], in_=p_h0[:])
        finish_h(0, h_all, snei_col)

        # ---------------- GNN layers ----------------
        for d in range(DEPTH):
            CORES = v["n_cores"]
            G_sb = big.tile([P, CORES, IT, PAY], BF16, tag="G_sb")
            snei_f = big.tile([P, CORES, IT], FP32, tag="snei_f")
            for c0 in range(CORES):
                nc.sync.dma_start(
                    out=G_sb[:, c0],
                    in_=Gs[d][c0 * P:(c0 + 1) * P, :].rearrange(
                        "p (t c) -> p t c", t=IT))
                nc.vector.tensor_copy(out=snei_f[:, c0],
                                      in_=G_sb[:, c0, :, H])

            # s_self row (free-dim orientation) for this layer
            colv = s_cols[d]
            srow = work.tile([1, S], FP32, tag="srow", bufs=1)
            for it in range(IT):
                p_sc = pcyc.tile([1, P], FP32, tag="ps")
                nc.tensor.matmul(p_sc[:], lhsT=colv[:, it:it + 1],
                                 rhs=ident_f[:], start=True, stop=True)
                nc.any.tensor_copy(out=srow[0:1, it * P:(it + 1) * P],
                                   in_=p_sc[:])
            pre_t = big.tile([P, S], FP16, tag="pre_t")
            for it in range(IT):
                p_bc = pcyc.tile([P, P], FP32, tag="ps")
                nc.tensor.matmul(p_bc[:], lhsT=ones_f[0:1, :],
                                 rhs=srow[0:1, it * P:(it + 1) * P],
                                 start=True, stop=True)
                nc.scalar.activation(pre_t[:, it * P:(it + 1) * P],
                                     p_bc[:], AF.Identity, bias=0.0,
                                     scale=1.0 / MASK)

            # psum accumulators for agg^T
            p_out = [[pacc.tile([P, IC], FP32, name=f"pout_{d}_{hc}_{ic}",
                                tag="pout", bufs=HC * NIC)
                      for ic in range(NIC)] for hc in range(HC)]

            if d == 0:
                # fused: load A -> PE transpose -> C'=A^T-1 -> store + layer-0 use
                for jh in range(NJH):
                    for ic in range(NIC):
                        a_sb = aload.tile([P, IC // P, ACH], BF16, tag="a_sb")
                        nc.gpsimd.dma_start(
                            out=a_sb[:],
                            in_=A[ic * IC:(ic + 1) * IC,
                                  jh * ACH:(jh + 1) * ACH].rearrange(
                                      "(b p) j -> p b j", p=P))
                        TB = min(4, JL)
                        for jl in range(JL):
                            jt = jh * JL + jl
                            if jl % TB == 0:
                                stage = work.tile([P, TB, IC], FP16,
                                                  tag="t_stage", bufs=2)
                            p_tr = pcyc.tile([P, IC], FP32, tag="ps")
                            for b in range(IC // P):
                                nc.tensor.matmul(
                                    p_tr[:, b * P:(b + 1) * P],
                                    lhsT=a_sb[:, b, jl * P:(jl + 1) * P],
                                    rhs=ident_bf[:], start=True, stop=True)
                            t_sb = stage[:, jl % TB, :]
                            nc.vector.tensor_scalar_add(t_sb, p_tr[:],
                                                        neg1_col[:, 0:1])
                            t2 = work.tile([P, IC], FP16, tag="t2")
                            nc.vector.tensor_tensor(
                                t2[:], t_sb,
                                pre_t[:, ic * IC:(ic + 1) * IC], ALU.add)
                            m_sb = work.tile([P, IC], BF16, tag="m_sb")
                            nc.scalar.activation(
                                m_sb[:], t2[:], AF.Sigmoid,
                                bias=snei_f[:, jt // IT, jt % IT:jt % IT + 1],
                                scale=MASK)
                            for hc in range(HC):
                                nc.tensor.matmul(
                                    p_out[hc][ic][:],
                                    lhsT=G_sb[:, jt // IT, jt % IT,
                                              hc * P:(hc + 1) * P],
                                    rhs=m_sb[:],
                                    start=(jh == 0 and jl == 0),
                                    stop=(jh == NJH - 1 and jl == JL - 1))
                            if jl % TB == TB - 1:
                                nc.sync.dma_start(
                                    out=T0[:, jt - TB + 1:jt + 1,
                                           ic * IC:(ic + 1) * IC],
                                    in_=stage[:])
            else:
                TBL = min(4, JT)
                for jb in range(JT // TBL):
                    t_ld = work.tile([P, TBL, S], FP16, tag="t_ld", bufs=2)
                    nc.sync.dma_start(
                        out=t_ld[:],
                        in_=T0[:, jb * TBL:(jb + 1) * TBL, :])
                    t_adj = work.tile([P, TBL, S], FP16, tag="t_adj", bufs=2)
                    nc.vector.tensor_tensor(
                        t_adj[:], t_ld[:],
                        pre_t[:, None, :].to_broadcast([P, TBL, S]), ALU.add)
                    for jl in range(TBL):
                        jt = jb * TBL + jl
                        m_sb2 = work.tile([P, S], BF16, tag="m_sb2", bufs=3)
                        nc.scalar.activation(
                            m_sb2[:], t_adj[:, jl, :], AF.Sigmoid,
                            bias=snei_f[:, jt // IT, jt % IT:jt % IT + 1],
                            scale=MASK)
                        for hc in range(HC):
                            for ic in range(NIC):
                                nc.tensor.matmul(
                                    p_out[hc][ic][:],
                                    lhsT=G_sb[:, jt // IT, jt % IT,
                                              hc * P:(hc + 1) * P],
                                    rhs=m_sb2[:, ic * IC:(ic + 1) * IC],
                                    start=(jt == 0), stop=(jt == JT - 1))

            # agg^T -> SBUF (bf16)
            aggT = big.tile([P, HC, S], FP32, tag="aggT")
            for hc in range(HC):
                for ic in range(NIC):
                    nc.any.tensor_copy(out=aggT[:, hc, ic * IC:(ic + 1) * IC],
                                       in_=p_out[hc][ic][:])

            # h_{d+1} = relu(agg @ W_node[d] + b_node[d])
            h_all = big.tile([P, IT, PAY], BF16, tag="h_all")
            snei_col = big.tile([P, IT], FP32, tag="snei_col")
            for it in range(IT):
                p_h = pcyc.tile([P, H], FP32, tag="ps")
                for kc in range(HC):
                    nc.tensor.matmul(p_h[:],
                                     lhsT=aggT[:, kc, it * P:(it + 1) * P],
                                     rhs=wnode_sb[:, d, kc, :],
                                     start=(kc == 0), stop=False)
                nc.tensor.matmul(p_h[:], lhsT=ones_f[0:1, :],
                                 rhs=bnode_f[0:1, d * H:(d + 1) * H],
                                 start=False, stop=True)
                nc.scalar.activation(h_all[:, it, 0:H], p_h[:], AF.Relu)
            finish_h(d + 1, h_all, snei_col)

        # ---------------- readout ----------------
        p_g = pacc.tile([1, H], FP32, name="p_g", tag="p_g", bufs=1)
        for it in range(IT):
            nc.tensor.matmul(p_g[:], lhsT=ones_bf[:, 0:1],
                             rhs=h_all[:, it, 0:H],
                             start=(it == 0), stop=(it == IT - 1))
        gpart = work.tile([1, H], FP32, tag="gpart")
        nc.any.tensor_copy(out=gpart[:], in_=p_g[:])
        nc.sync.dma_start(out=ar_in[:], in_=gpart[:])
        if v["no_collectives"]:
            nc.sync.dma_start(out=ar_out[:], in_=ar_in[:])
        else:
            nc.gpsimd.collective_compute("AllReduce", ALU.add, replica_groups=rg,
                                         ins=[ar_in[:].opt()],
                                         outs=[ar_out[:].opt()])
        gsum = work.tile([1, H], FP32, tag="gsum")
        nc.sync.dma_start(out=gsum[:], in_=ar_out[:])

        def row_to_col(row_sb, width, out_col, col_idx):
            """scatter a [1, width] fp32 row onto partitions as [width, 1]"""
            for c in range((width + P - 1) // P):
                w = min(P, width - c * P)
                p_c = pcyc.tile([P, 1], FP32, tag="ps")
                nc.tensor.matmul(p_c[0:w, :],
                                 lhsT=row_sb[0:1, c * P:c * P + w],
                                 rhs=ones_f[0:1, 0:1], start=True, stop=True)
                nc.any.tensor_copy(out=out_col[0:w, col_idx + c:col_idx + c + 1],
                                   in_=p_c[0:w, :])

        gcat = work.tile([P, 2 + (MF + P - 1) // P], FP32, tag="gcat")
        nc.vector.memset(gcat[:], 0.0)
        row_to_col(gsum, H, gcat, 0)
        row_to_col(mol_sb, MF, gcat, HC)
        kdims = [P, P] + [min(P, MF - c * P) for c in range((MF + P - 1) // P)]

        def mlp_col(col_tile, kd, w_sb, b_row):
            """column-in / column-out MLP layer: stays on partitions, so no
            transposes between layers.  out[n] = relu(sum_k W[k,n] g[k] + b[n])
            via lhsT=W k-n chunks, rhs=g column chunks."""
            p_c2 = pcyc.tile([P, RT], FP32, tag="ps")
            for nch in range(RT):
                for kt, kw in enumerate(kd):
                    nc.tensor.matmul(
                        p_c2[:, nch:nch + 1],
                        lhsT=w_sb[0:kw, kt, nch * P:(nch + 1) * P],
                        rhs=col_tile[0:kw, kt:kt + 1],
                        start=(kt == 0), stop=False)
                nc.tensor.matmul(
                    p_c2[:, nch:nch + 1],
                    lhsT=b_row[0:1, nch * P:(nch + 1) * P],
                    rhs=ones_f[0:1, 0:1], start=False, stop=True)
            g_col = work.tile([P, RT], FP32, tag="gcol")
            nc.scalar.activation(g_col[:], p_c2[:], AF.Relu)
            return g_col

        gcol = mlp_col(gcat, kdims, wro_sb, broin_sb[:])
        for d2 in range(N_RO):
            gcol = mlp_col(gcol, [P] * RT, wroh_sb[:, d2],
                           broh_sb[0:1, d2 * RH:(d2 + 1) * RH])
        p_o = pcyc.tile([1, 1], FP32, tag="ps")
        for kt in range(RT):
            nc.tensor.matmul(p_o[:], lhsT=gcol[:, kt:kt + 1],
                             rhs=wout_sb[:, kt, :], start=(kt == 0), stop=False)
        nc.tensor.matmul(p_o[:], lhsT=ones_f[0:1, 0:1], rhs=bout_sb[:],
                         start=False, stop=True)
        o_sb = work.tile([1, 1], FP32, tag="o_sb")
        nc.any.tensor_copy(out=o_sb[:], in_=p_o[:])
        nc.sync.dma_start(out=OUT[:], in_=o_sb[:])


# ---------------------------------------------------------------------------
# host-side wrapper
# ---------------------------------------------------------------------------

_BUILT = {}


def _get(config):
    if config not in _BUILT:
        _BUILT[config] = build_gcnn(*config)
    return _BUILT[config]


def make_in_maps(inputs, N=8192, F=133, H=256, MF=200, RH=512, DEPTH=3,
                 N_RO=2, n_cores=8):
    S = N // n_cores
    f32 = lambda x: np.ascontiguousarray(np.asarray(x, dtype=np.float32))
    A = f32(inputs["adjacency_matrix"])
    X = f32(inputs["atom_feature_matrix"])
    base = {
        "mol": f32(inputs["molecule_features_vector"]).reshape(1, MF),
        "w_in": f32(inputs["W_in"]),
        "b_in": f32(inputs["b_in"]).reshape(1, H),
        "w_att": f32(inputs["W_att"]).reshape(2, H),
        "b_att": f32(inputs["b_att"]).reshape(1, 1),
        "w_node": f32(inputs["W_node"]),
        "b_node": f32(inputs["b_node"]).reshape(1, DEPTH * H),
        "w_ro_in": f32(inputs["W_ro_in"]),
        "b_ro_in": f32(inputs["b_ro_in"]).reshape(1, RH),
        "w_ro_hid": f32(inputs["W_ro_hid"]),
        "b_ro_hid": f32(inputs["b_ro_hid"]).reshape(1, N_RO * RH),
        "w_out": f32(inputs["W_out"]).reshape(RH, 1),
        "b_out": f32(inputs["b_out"]).reshape(1, 1),
    }
    return [dict(base,
                 adj=np.ascontiguousarray(A[c * S:(c + 1) * S]),
                 atom=np.ascontiguousarray(X[c * S:(c + 1) * S]))
            for c in range(n_cores)]


def run(inputs, N=8192, F=133, H=256, MF=200, RH=512, DEPTH=3, N_RO=2,
        n_cores=8, ach=None, ic=None, **spmd_kwargs):
    nc = _get((N, F, H, MF, RH, DEPTH, N_RO, n_cores, ach, ic))
    in_maps = make_in_maps(inputs, N, F, H, MF, RH, DEPTH, N_RO, n_cores)
    res = run_bass_kernel_spmd(nc, in_maps, core_ids=list(range(n_cores)),
                               **spmd_kwargs)
    out = np.asarray(res.results[0]["out"], dtype=np.float32).reshape(())
    return out, res


def kernel(**inputs):
    out, _ = run(inputs)
    return out
